# revision 16
# baseline (speedup 1.0000x reference)
"""GNN message-passing kernel for Trainium2 (8 NeuronCores, Bass/Tile).

 - Host: sort edges by dst; shard 128 graphs (and node/edge ranges) per core.
   Nodes go in blocks of 32; each core's edges for a block are packed into
   128-slot tiles (runs never split). Tiles-per-block = max over cores, so a
   single SPMD program has a shared static schedule; per-core variation lives
   in input data only.
 - Device: feature-major stacked MLP on PE (block-diagonal weights, fp32r),
   on-device dist, PE transpose to edge-major, one-hot window aggregation
   into PSUM node stripes, then relu/softmax/pooling/z.
"""

import sys

sys.path.insert(0, "/opt/trn_rl_repo")

from contextlib import ExitStack

import numpy as np

import concourse.bass as bass
import concourse.tile as tile
from concourse import bacc, mybir
from concourse.bass_utils import run_bass_kernel_spmd
from concourse.masks import make_identity

AF = mybir.ActivationFunctionType
ALU = mybir.AluOpType
DT = mybir.dt

N_NODES = 262144
N_EDGES = 4194304
N_GRAPHS = 1024
N_CORES = 8
G_PER_CORE = N_GRAPHS // N_CORES

TILE_E = 128
BLK = 32
CHUNK = 512
NSTACK = 8
SUPER = CHUNK * NSTACK
TILES_PER_SUPER = SUPER // TILE_E
STRIPE = 512
BLKS_PER_STRIPE = STRIPE // BLK
GWIN = 160


def _host_prep(x, pos, edge_index, batch):
    src = np.asarray(edge_index[0], dtype=np.int64)
    dst = np.asarray(edge_index[1], dtype=np.int64)
    batch = np.asarray(batch, dtype=np.int64)
    n = x.shape[0]

    order = np.argsort(dst, kind="stable")
    dsts = dst[order]
    srcs = src[order]

    gstart = np.searchsorted(batch, np.arange(N_GRAPHS + 1))
    core_node_lo = gstart[np.arange(N_CORES) * G_PER_CORE]
    core_node_hi = gstart[(np.arange(N_CORES) + 1) * G_PER_CORE]
    core_edge_lo = np.searchsorted(dsts, core_node_lo)
    core_edge_hi = np.searchsorted(dsts, core_node_hi)

    deg_all = np.bincount(dsts, minlength=n)

    nl_max = int((core_node_hi - core_node_lo).max())
    nl_pad = -(-nl_max // STRIPE) * STRIPE
    n_blocks = nl_pad // BLK

    core_rows = []
    tiles_needed = np.zeros((N_CORES, n_blocks), dtype=np.int64)
    for c in range(N_CORES):
        nlo, nhi = int(core_node_lo[c]), int(core_node_hi[c])
        nloc = nhi - nlo
        deg = np.zeros(nl_pad, dtype=np.int64)
        deg[:nloc] = deg_all[nlo:nhi]
        runstart = np.zeros(nl_pad, dtype=np.int64)
        np.cumsum(deg[:-1], out=runstart[1:])
        core_rows.append((nlo, nloc, deg, runstart))
        dblk = deg.reshape(n_blocks, BLK)
        for b in range(n_blocks):
            d = dblk[b]
            used, t = 0, 0
            for dv in d:
                if dv == 0:
                    continue
                if t == 0:
                    t = 1
                if used + dv > TILE_E:
                    t += 1
                    used = 0
                used += dv
            tiles_needed[c, b] = t

    tpb = tiles_needed.max(axis=0)
    tile_block = np.repeat(np.arange(n_blocks), tpb)
    n_tiles = len(tile_block)
    n_tiles_pad = -(-n_tiles // TILES_PER_SUPER) * TILES_PER_SUPER
    tile_block = np.concatenate(
        [tile_block, np.full(n_tiles_pad - n_tiles, n_blocks - 1, dtype=np.int64)]
    )
    e_pad = n_tiles_pad * TILE_E
    n_super = e_pad // SUPER
    ncols = e_pad // NSTACK
    tile_start = np.zeros(n_blocks + 1, dtype=np.int64)
    np.cumsum(tpb, out=tile_start[1:])

    xp = np.zeros((n, 8), dtype=np.float32)
    xp[:, 0:4] = x
    xp[:, 4:7] = pos

    s_all = np.arange(e_pad)
    col_of_slot = (s_all // SUPER) * CHUNK + (s_all % CHUNK)
    ch_of_slot = (s_all % SUPER) // CHUNK

    core_inputs = []
    for c in range(N_CORES):
        nlo, nloc, deg, runstart = core_rows[c]
        elo = int(core_edge_lo[c])
        e_loc = int(core_edge_hi[c]) - elo
        cs = srcs[elo : elo + e_loc]
        cd = dsts[elo : elo + e_loc] - nlo

        node_slot = np.full(nl_pad, 0, dtype=np.int64)
        for b in range(n_blocks):
            ti = tile_start[b]
            used = 0
            for nn in range(b * BLK, (b + 1) * BLK):
                dv = deg[nn]
                if dv == 0:
                    continue
                if used + dv > TILE_E:
                    ti += 1
                    used = 0
                node_slot[nn] = ti * TILE_E + used
                used += dv

        rank = np.arange(e_loc) - runstart[cd]
        slot = node_slot[cd] + rank

        sl_src = np.zeros(e_pad, dtype=np.int64)
        sl_dstl = np.zeros(e_pad, dtype=np.int64)
        sl_real = np.zeros(e_pad, dtype=bool)
        sl_src[slot] = cs
        sl_dstl[slot] = cd
        sl_real[slot] = True

        sl_recip = np.zeros(e_pad, dtype=np.float32)
        sl_recip[slot] = (1.0 / np.maximum(deg[cd], 1)).astype(np.float32)

        tid = s_all // TILE_E
        sl_key = np.where(sl_real, sl_dstl - tile_block[tid] * BLK, 0).astype(
            np.uint8
        )

        g_src = xp[sl_src]
        g_dst = xp[np.minimum(sl_dstl + nlo, n - 1)]
        g_src[~sl_real] = 0.0
        g_dst[~sl_real] = 0.0

        m_pre = np.zeros((64, ncols), dtype=np.float32)
        for r in range(4):
            m_pre[8 * ch_of_slot + r, col_of_slot] = g_dst[:, r]
            m_pre[8 * ch_of_slot + 4 + r, col_of_slot] = g_src[:, r]

        rel_pre = np.zeros((24, ncols), dtype=np.float32)
        relv = g_src[:, 4:7] - g_dst[:, 4:7]
        for r in range(3):
            rel_pre[3 * ch_of_slot + r, col_of_slot] = relv[:, r]

        em_col = s_all // TILE_E
        em_row = s_all % TILE_E
        keys_em = np.zeros((TILE_E, n_tiles_pad), dtype=np.float32)
        keys_em[em_row, em_col] = sl_key
        recip_em = np.zeros((TILE_E, n_tiles_pad), dtype=np.float32)
        recip_em[em_row, em_col] = sl_recip

        brel = np.full(nl_pad, GWIN - 1, dtype=np.uint8)
        brel[:nloc] = (batch[nlo : nlo + nloc] - c * G_PER_CORE).astype(np.uint8)
        brel_nm = np.zeros((128, nl_pad // 128), dtype=np.float32)
        brel_nm[np.arange(nl_pad) % 128, np.arange(nl_pad) // 128] = brel

        cu8 = np.zeros((128, BLK + GWIN + nl_pad // 128), dtype=np.float32)
        cu8[:, 0:BLK] = np.tile(np.arange(BLK, dtype=np.float32), (128, 1))
        cu8[:, BLK : BLK + GWIN] = np.tile(
            np.arange(GWIN, dtype=np.float32), (128, 1)
        )
        cu8[:, BLK + GWIN :] = brel_nm
        core_inputs.append(
            dict(
                m_pre=m_pre,
                rel_pre=rel_pre,
                keys_em=keys_em,
                recip_em=recip_em,
                const_u8=cu8,
            )
        )

    meta = dict(
        e_pad=e_pad,
        n_super=n_super,
        n_tiles_pad=n_tiles_pad,
        nl_pad=nl_pad,
        tile_block=tile_block,
        core_bounds=(core_node_lo, core_node_hi),
    )
    return core_inputs, meta


def _weight_inputs(W1, b1, W2, b2, Wp, bp, Wz, bz):
    W1 = np.asarray(W1, np.float32)
    W2 = np.asarray(W2, np.float32)
    W1xbd = np.zeros((64, 128), dtype=np.float32)
    W1dbd = np.zeros((8, 128), dtype=np.float32)
    W2bd = np.zeros((128, 128), dtype=np.float32)
    b1bd = np.zeros((128, 1), dtype=np.float32)
    b2bd = np.zeros((128, 1), dtype=np.float32)
    for c in range(NSTACK):
        W1xbd[8 * c : 8 * c + 8, 16 * c : 16 * c + 16] = W1[0:8]
        W1dbd[c, 16 * c : 16 * c + 16] = W1[8]
        W2bd[16 * c : 16 * c + 16, 16 * c : 16 * c + 16] = W2
        b1bd[16 * c : 16 * c + 16, 0] = b1
        b2bd[16 * c : 16 * c + 16, 0] = b2
    onesbd = np.zeros((24, 8), dtype=np.float32)
    for c in range(NSTACK):
        onesbd[3 * c : 3 * c + 3, c] = 1.0
    iota32 = np.tile(np.arange(BLK, dtype=np.uint8), (128, 1))
    iota_g = np.tile(np.arange(GWIN, dtype=np.uint8), (128, 1))
    Wzb = np.zeros((33, 8), dtype=np.float32)
    Wzb[:32] = np.asarray(Wz, np.float32)
    Wzb[32] = np.asarray(bz, np.float32)
    Wpd = np.asarray(Wp, np.float32)[:, 0:1] - np.asarray(Wp, np.float32)[:, 1:2]
    # pack: f32r block [128, 256+8]: W1x(0:128), W2(128:256), ones(256:264),
    #       W1d rows live in partitions 64:72 of W1x cols... keep separate cols:
    cr = np.zeros((128, 264 + 128), dtype=np.float32)
    cr[0:64, 0:128] = W1xbd
    cr[0:128, 128:256] = W2bd
    cr[0:24, 256:264] = onesbd
    cr[0:8, 264:392] = W1dbd
    cf = np.zeros((128, 2 + 1 + 8), dtype=np.float32)
    cf[:, 0:1] = b1bd
    cf[:, 1:2] = b2bd
    cf[0:16, 2:3] = Wpd
    cf[0:33, 3:11] = Wzb
    w = dict(const_f32r=cr, const_f32=cf)
    return w, float(np.asarray(bp)[0] - np.asarray(bp)[1])


def _build_program(meta, bp_diff):
    e_pad = meta["e_pad"]
    n_super = meta["n_super"]
    n_tiles_pad = meta["n_tiles_pad"]
    nl_pad = meta["nl_pad"]
    tile_block = meta["tile_block"]
    ncols = e_pad // NSTACK
    n_stripes = nl_pad // STRIPE
    n_ntile = nl_pad // 128

    f32, f32r, bf16, u8 = DT.float32, DT.float32r, DT.bfloat16, DT.uint8

    tile_stripe = tile_block // BLKS_PER_STRIPE
    stripe_last = (
        np.searchsorted(tile_stripe, np.arange(n_stripes), "right") - 1
    )

    nc = bacc.Bacc(None, target_bir_lowering=False, debug=False)

    d_m = nc.dram_tensor("m_pre", [64, ncols], f32r, kind="ExternalInput")
    d_rel = nc.dram_tensor("rel_pre", [24, ncols], f32, kind="ExternalInput")
    d_keys = nc.dram_tensor("keys_em", [TILE_E, n_tiles_pad], f32, kind="ExternalInput")
    d_recip = nc.dram_tensor(
        "recip_em", [TILE_E, n_tiles_pad], f32, kind="ExternalInput"
    )
    d_cr = nc.dram_tensor("const_f32r", [128, 392], f32r, kind="ExternalInput")
    d_cf = nc.dram_tensor("const_f32", [128, 11], f32, kind="ExternalInput")
    d_cu8 = nc.dram_tensor(
        "const_u8", [128, BLK + GWIN + n_ntile], f32, kind="ExternalInput"
    )

    d_s = nc.dram_tensor("s_out", [128, n_ntile, 2], f32, kind="ExternalOutput")
    d_z = nc.dram_tensor("z_out", [GWIN, 8], f32, kind="ExternalOutput")

    with tile.TileContext(nc) as tc, ExitStack() as ctx:
        const = ctx.enter_context(tc.tile_pool(name="const", bufs=1))
        sb = ctx.enter_context(tc.tile_pool(name="sb", bufs=3))
        sbe = ctx.enter_context(tc.tile_pool(name="sbe", bufs=4))
        sbn = ctx.enter_context(tc.tile_pool(name="sbn", bufs=2))
        persist = ctx.enter_context(tc.tile_pool(name="persist", bufs=1))
        ps_h1 = ctx.enter_context(tc.tile_pool(name="psh1", bufs=1, space="PSUM"))
        ps_msg = ctx.enter_context(tc.tile_pool(name="psmsg", bufs=1, space="PSUM"))
        ps_tr = ctx.enter_context(tc.tile_pool(name="pstr", bufs=2, space="PSUM"))
        ps_stripe = ctx.enter_context(tc.tile_pool(name="pss", bufs=2, space="PSUM"))
        ps_pool = ctx.enter_context(tc.tile_pool(name="psp", bufs=1, space="PSUM"))
        ps_misc = ctx.enter_context(tc.tile_pool(name="psm", bufs=1, space="PSUM"))

        c_r = const.tile([128, 392], f32r)
        c_f = const.tile([128, 11], f32)
        c_u = const.tile([128, BLK + GWIN + n_ntile], f32)
        nc.sync.dma_start(out=c_r[:], in_=d_cr[:])
        nc.sync.dma_start(out=c_f[:], in_=d_cf[:])
        nc.sync.dma_start(out=c_u[:], in_=d_cu8[:])
        cW1x = c_r[0:64, 0:128]
        cW2 = c_r[0:128, 128:256]
        cones = c_r[0:24, 256:264]
        cW1d = c_r[0:8, 264:392]
        cb1 = c_f[:, 0:1]
        cb2 = c_f[:, 1:2]
        cWp = c_f[0:16, 2:3]
        cWzb = c_f[0:33, 3:11]
        cio32 = c_u[:, 0:BLK]
        ciog = c_u[:, BLK : BLK + GWIN]
        cbrel = c_u[:, BLK + GWIN :]
        cident = const.tile([128, 128], f32)
        make_identity(nc, cident[:])
        czero16 = const.tile([1, 16], f32)
        czero512 = const.tile([1, CHUNK], f32)
        nc.gpsimd.memset(czero16[:], 0.0)
        nc.gpsimd.memset(czero512[:], 0.0)

        h_nm = persist.tile([128, n_ntile, 16], f32)
        s_nm = persist.tile([128, n_ntile, 2], f32)

        pooledT = ps_pool.tile([32, GWIN], f32)
        stripe_tiles = {}

        def open_stripe(s):
            pt = ps_stripe.tile([16, STRIPE], f32, tag="stripe")
            stripe_tiles[s] = pt
            empty = stripe_last[s] < 0 or (
                s > 0 and stripe_last[s] == stripe_last[s - 1]
            )
            nc.tensor.matmul(
                out=pt[:], lhsT=czero16[:], rhs=czero512[:],
                start=True, stop=bool(empty), skip_group_check=True,
            )

        def close_stripe(s):
            pt = stripe_tiles.pop(s)
            hT = sbn.tile([16, STRIPE], f32, tag="hT")
            nc.scalar.activation(out=hT[:], in_=pt[:], func=AF.Relu)
            node_phase(s, hT)

        def node_phase(s, hT):
            lg = ps_misc.tile([1, STRIPE], f32, tag="misc")
            nc.tensor.matmul(
                out=lg[:], lhsT=cWp, rhs=hT[:],
                start=True, stop=True,
            )
            s0 = sbn.tile([1, STRIPE], f32, tag="s0")
            nc.scalar.activation(
                out=s0[:], in_=lg[:], func=AF.Sigmoid, bias=float(bp_diff)
            )
            for j in range(STRIPE // 128):
                nt = s * (STRIPE // 128) + j
                ph = ps_misc.tile([128, 16], f32, tag="misc")
                nc.tensor.transpose(
                    out=ph[:], in_=hT[:, j * 128 : (j + 1) * 128],
                    identity=cident[:16, :16],
                )
                nc.vector.tensor_copy(out=h_nm[:, nt, :], in_=ph[:])
                psn = ps_misc.tile([128, 1], f32, tag="misc")
                nc.tensor.transpose(
                    out=psn[:], in_=s0[:, j * 128 : (j + 1) * 128],
                    identity=cident[:1, :1],
                )
                nc.vector.tensor_copy(out=s_nm[:, nt, 0:1], in_=psn[:])
                nc.scalar.activation(
                    out=s_nm[:, nt, 1:2], in_=s_nm[:, nt, 0:1],
                    func=AF.Copy, scale=-1.0, bias=1.0,
                )
                outer = sbn.tile([128, 32], bf16, tag="outer")
                nc.vector.tensor_tensor(
                    out=outer[:, 0:16],
                    in0=h_nm[:, nt, :],
                    in1=s_nm[:, nt, 0:1].to_broadcast([128, 16]),
                    op=ALU.mult,
                )
                nc.vector.tensor_tensor(
                    out=outer[:, 16:32],
                    in0=h_nm[:, nt, :],
                    in1=s_nm[:, nt, 1:2].to_broadcast([128, 16]),
                    op=ALU.mult,
                )
                og = sbn.tile([128, GWIN], bf16, tag="og")
                nc.vector.tensor_tensor(
                    out=og[:],
                    in0=cbrel[:, nt : nt + 1].to_broadcast([128, GWIN]),
                    in1=ciog,
                    op=ALU.is_equal,
                )
                nc.tensor.matmul(
                    out=pooledT[:], lhsT=outer[:], rhs=og[:],
                    start=(nt == 0), stop=(nt == n_ntile - 1),
                    skip_group_check=True,
                )

        # ---------------- edge phase ----------------
        cur_stripe = 0
        open_stripe(0)

        for sp in range(n_super):
            mt = sb.tile([64, CHUNK], f32r, tag="m")
            nc.sync.dma_start(out=mt[:], in_=d_m[:, sp * CHUNK : (sp + 1) * CHUNK])
            rt = sb.tile([24, CHUNK], f32, tag="rel")
            nc.sync.dma_start(
                out=rt[:], in_=d_rel[:, sp * CHUNK : (sp + 1) * CHUNK]
            )
            kt = sb.tile([TILE_E, TILES_PER_SUPER], f32, tag="keys")
            nc.sync.dma_start(
                out=kt[:],
                in_=d_keys[:, sp * TILES_PER_SUPER : (sp + 1) * TILES_PER_SUPER],
            )
            rct = sb.tile([TILE_E, TILES_PER_SUPER], f32, tag="recip")
            nc.sync.dma_start(
                out=rct[:],
                in_=d_recip[:, sp * TILES_PER_SUPER : (sp + 1) * TILES_PER_SUPER],
            )

            sq = sb.tile([24, CHUNK], f32r, tag="sq")
            nc.vector.tensor_tensor(out=sq[:], in0=rt[:], in1=rt[:], op=ALU.mult)
            d2 = ps_misc.tile([8, CHUNK], f32, tag="misc")
            nc.tensor.matmul(
                out=d2[:], lhsT=cones, rhs=sq[:],
                start=True, stop=True,
            )
            dist = sb.tile([8, CHUNK], f32r, tag="dist")
            nc.scalar.activation(out=dist[:], in_=d2[:], func=AF.Sqrt)

            h1p = ps_h1.tile([128, CHUNK], f32, tag="h1")
            nc.tensor.matmul(
                out=h1p[:], lhsT=cW1x, rhs=mt[:],
                start=True, stop=False, skip_group_check=True,
            )
            nc.tensor.matmul(
                out=h1p[:], lhsT=cW1d, rhs=dist[:],
                start=False, stop=True, skip_group_check=True,
            )
            h1 = sb.tile([128, CHUNK], f32r, tag="h1s")
            nc.scalar.activation(out=h1[:], in_=h1p[:], func=AF.Silu, bias=cb1)
            msgp = ps_msg.tile([128, CHUNK], f32, tag="msg")
            nc.tensor.matmul(
                out=msgp[:], lhsT=cW2, rhs=h1[:],
                start=True, stop=True,
            )
            msgT = sb.tile([128, CHUNK], f32, tag="msgT")
            nc.vector.tensor_tensor(
                out=msgT[:], in0=msgp[:],
                in1=cb2.to_broadcast([128, CHUNK]), op=ALU.add,
            )

            oh = sb.tile([TILE_E, TILES_PER_SUPER, BLK], bf16, tag="oh")
            nc.vector.tensor_tensor(
                out=oh[:],
                in0=kt[:].unsqueeze(2).to_broadcast(
                    [TILE_E, TILES_PER_SUPER, BLK]
                ),
                in1=cio32.unsqueeze(1).to_broadcast(
                    [TILE_E, TILES_PER_SUPER, BLK]
                ),
                op=ALU.is_equal,
            )

            msge_of_b = []
            for b in range(4):
                trp = ps_tr.tile([128, 128], f32, tag="tr")
                nc.tensor.transpose(
                    out=trp[:], in_=msgT[:, b * 128 : (b + 1) * 128],
                    identity=cident[:],
                )
                msge = sbe.tile([128, NSTACK, 16], bf16, tag="msge")
                nc.vector.tensor_tensor(
                    out=msge[:],
                    in0=trp[:].rearrange("p (a b) -> p a b", a=NSTACK),
                    in1=rct[:, b : TILES_PER_SUPER : 4]
                    .unsqueeze(2)
                    .to_broadcast([TILE_E, NSTACK, 16]),
                    op=ALU.mult,
                )
                msge_of_b.append(msge)

            for cch in range(NSTACK):
                for b in range(4):
                    t_loc = cch * 4 + b
                    t_glob = sp * TILES_PER_SUPER + t_loc
                    s_of_t = int(tile_stripe[t_glob])
                    while cur_stripe < s_of_t:
                        close_stripe(cur_stripe)
                        cur_stripe += 1
                        open_stripe(cur_stripe)
                    blk = int(tile_block[t_glob])
                    coff = (blk % BLKS_PER_STRIPE) * BLK
                    pt = stripe_tiles[cur_stripe]
                    nc.tensor.matmul(
                        out=pt[:, coff : coff + BLK],
                        lhsT=msge_of_b[b][:, cch, :],
                        rhs=oh[:, t_loc, :],
                        start=False,
                        stop=bool(t_glob == int(stripe_last[s_of_t])),
                        skip_group_check=True,
                    )

        close_stripe(cur_stripe)
        for s2 in range(cur_stripe + 1, n_stripes):
            open_stripe(s2)
            close_stripe(s2)

        # ---------------- outputs ----------------
        pooled_sb = sbn.tile([33, GWIN], f32, tag="pooled")
        nc.gpsimd.memset(pooled_sb[:], 1.0)
        nc.vector.tensor_copy(out=pooled_sb[0:32, :], in_=pooledT[:])
        zp = ps_misc.tile([128, 8], f32, tag="misc")
        nc.tensor.matmul(
            out=zp[:],
            lhsT=pooled_sb[:, 0:128],
            rhs=cWzb,
            start=True, stop=True,
        )
        zs = sbn.tile([128, 8], f32, tag="zs")
        nc.vector.tensor_copy(out=zs[:], in_=zp[:])
        nc.sync.dma_start(out=d_z[0:128, :], in_=zs[:])
        zp2 = ps_misc.tile([32, 8], f32, tag="misc")
        nc.tensor.matmul(
            out=zp2[:],
            lhsT=pooled_sb[:, 128:GWIN],
            rhs=cWzb,
            start=True, stop=True,
        )
        zs2 = sbn.tile([32, 8], f32, tag="zs2")
        nc.vector.tensor_copy(out=zs2[:], in_=zp2[:])
        nc.sync.dma_start(out=d_z[128:GWIN, :], in_=zs2[:])

        nc.sync.dma_start(out=d_s[:], in_=s_nm[:])

    nc.compile()
    return nc


LAST_RESULT = None
LAST_EXEC_WALL_S = None


def kernel(x, pos, W1, b1, W2, b2, Wp, bp, Wz, bz, edge_index, batch):
    x = np.asarray(x, dtype=np.float32)
    pos = np.asarray(pos, dtype=np.float32)

    core_inputs, meta = _host_prep(x, pos, edge_index, batch)
    weights, bp_diff = _weight_inputs(W1, b1, W2, b2, Wp, bp, Wz, bz)

    nc = _build_program(meta, bp_diff)

    in_maps = []
    for c in range(N_CORES):
        m = dict(core_inputs[c])
        m.update(weights)
        in_maps.append(m)

    import time as _time

    t0 = _time.time()
    res = run_bass_kernel_spmd(nc, in_maps, list(range(N_CORES)))
    global LAST_RESULT, LAST_EXEC_WALL_S
    LAST_RESULT = res
    LAST_EXEC_WALL_S = _time.time() - t0
    results = res.results

    core_node_lo, core_node_hi = meta["core_bounds"]
    s_full = np.zeros((N_NODES, 2), dtype=np.float32)
    z_full = np.zeros((N_GRAPHS, 8), dtype=np.float32)
    for c in range(N_CORES):
        nlo, nhi = int(core_node_lo[c]), int(core_node_hi[c])
        nloc = nhi - nlo
        s_pc = results[c]["s_out"].transpose(1, 0, 2).reshape(-1, 2)
        s_full[nlo:nhi] = s_pc[:nloc]
        z_full[c * G_PER_CORE : (c + 1) * G_PER_CORE] = results[c]["z_out"][
            :G_PER_CORE
        ]
    return z_full, s_full


# revision 18
# speedup vs baseline: 1.2014x; 1.2014x over previous
"""GNN message-passing kernel for Trainium2 (8 NeuronCores, Bass/Tile).

 - Host: sort edges by dst; shard 128 graphs (and node/edge ranges) per core.
   Nodes go in blocks of 32; each core's edges for a block are packed into
   128-slot tiles (runs never split). Tiles-per-block = max over cores, so a
   single SPMD program has a shared static schedule; per-core variation lives
   in input data only.
 - Device: feature-major stacked MLP on PE (block-diagonal weights, fp32r),
   on-device dist, PE transpose to edge-major, one-hot window aggregation
   into PSUM node stripes, then relu/softmax/pooling/z.
"""

import sys

sys.path.insert(0, "/opt/trn_rl_repo")

from contextlib import ExitStack

import numpy as np

import concourse.bass as bass
import concourse.tile as tile
from concourse import bacc, mybir
from concourse.bass_utils import run_bass_kernel_spmd
from concourse.masks import make_identity

AF = mybir.ActivationFunctionType
ALU = mybir.AluOpType
DT = mybir.dt

N_NODES = 262144
N_EDGES = 4194304
N_GRAPHS = 1024
N_CORES = 8
G_PER_CORE = N_GRAPHS // N_CORES

TILE_E = 128
BLK = 32
CHUNK = 512
NSTACK = 8
SUPER = CHUNK * NSTACK
TILES_PER_SUPER = SUPER // TILE_E
STRIPE = 512
BLKS_PER_STRIPE = STRIPE // BLK
GWIN = 160


def _host_prep(x, pos, edge_index, batch):
    src = np.asarray(edge_index[0], dtype=np.int64)
    dst = np.asarray(edge_index[1], dtype=np.int64)
    batch = np.asarray(batch, dtype=np.int64)
    n = x.shape[0]

    order = np.argsort(dst, kind="stable")
    dsts = dst[order]
    srcs = src[order]

    gstart = np.searchsorted(batch, np.arange(N_GRAPHS + 1))
    core_node_lo = gstart[np.arange(N_CORES) * G_PER_CORE]
    core_node_hi = gstart[(np.arange(N_CORES) + 1) * G_PER_CORE]
    core_edge_lo = np.searchsorted(dsts, core_node_lo)
    core_edge_hi = np.searchsorted(dsts, core_node_hi)

    deg_all = np.bincount(dsts, minlength=n)

    nl_max = int((core_node_hi - core_node_lo).max())
    nl_pad = -(-nl_max // STRIPE) * STRIPE
    n_blocks = nl_pad // BLK

    core_rows = []
    tiles_needed = np.zeros((N_CORES, n_blocks), dtype=np.int64)
    for c in range(N_CORES):
        nlo, nhi = int(core_node_lo[c]), int(core_node_hi[c])
        nloc = nhi - nlo
        deg = np.zeros(nl_pad, dtype=np.int64)
        deg[:nloc] = deg_all[nlo:nhi]
        runstart = np.zeros(nl_pad, dtype=np.int64)
        np.cumsum(deg[:-1], out=runstart[1:])
        core_rows.append((nlo, nloc, deg, runstart))
        dblk = deg.reshape(n_blocks, BLK)
        for b in range(n_blocks):
            d = dblk[b]
            used, t = 0, 0
            for dv in d:
                if dv == 0:
                    continue
                if t == 0:
                    t = 1
                if used + dv > TILE_E:
                    t += 1
                    used = 0
                used += dv
            tiles_needed[c, b] = t

    tpb = tiles_needed.max(axis=0)
    tile_block = np.repeat(np.arange(n_blocks), tpb)
    n_tiles = len(tile_block)
    n_tiles_pad = -(-n_tiles // TILES_PER_SUPER) * TILES_PER_SUPER
    tile_block = np.concatenate(
        [tile_block, np.full(n_tiles_pad - n_tiles, n_blocks - 1, dtype=np.int64)]
    )
    e_pad = n_tiles_pad * TILE_E
    n_super = e_pad // SUPER
    ncols = e_pad // NSTACK
    tile_start = np.zeros(n_blocks + 1, dtype=np.int64)
    np.cumsum(tpb, out=tile_start[1:])

    xp = np.zeros((n, 8), dtype=np.float32)
    xp[:, 0:4] = x
    xp[:, 4:7] = pos

    s_all = np.arange(e_pad)
    col_of_slot = (s_all // SUPER) * CHUNK + (s_all % CHUNK)
    ch_of_slot = (s_all % SUPER) // CHUNK

    core_inputs = []
    for c in range(N_CORES):
        nlo, nloc, deg, runstart = core_rows[c]
        elo = int(core_edge_lo[c])
        e_loc = int(core_edge_hi[c]) - elo
        cs = srcs[elo : elo + e_loc]
        cd = dsts[elo : elo + e_loc] - nlo

        node_slot = np.full(nl_pad, 0, dtype=np.int64)
        for b in range(n_blocks):
            ti = tile_start[b]
            used = 0
            for nn in range(b * BLK, (b + 1) * BLK):
                dv = deg[nn]
                if dv == 0:
                    continue
                if used + dv > TILE_E:
                    ti += 1
                    used = 0
                node_slot[nn] = ti * TILE_E + used
                used += dv

        rank = np.arange(e_loc) - runstart[cd]
        slot = node_slot[cd] + rank

        sl_src = np.zeros(e_pad, dtype=np.int64)
        sl_dstl = np.zeros(e_pad, dtype=np.int64)
        sl_real = np.zeros(e_pad, dtype=bool)
        sl_src[slot] = cs
        sl_dstl[slot] = cd
        sl_real[slot] = True

        sl_recip = np.zeros(e_pad, dtype=np.float32)
        sl_recip[slot] = (1.0 / np.maximum(deg[cd], 1)).astype(np.float32)

        tid = s_all // TILE_E
        sl_key = np.where(sl_real, sl_dstl - tile_block[tid] * BLK, 0).astype(
            np.uint8
        )

        g_src = xp[sl_src]
        g_dst = xp[np.minimum(sl_dstl + nlo, n - 1)]
        g_src[~sl_real] = 0.0
        g_dst[~sl_real] = 0.0

        m_pre = np.zeros((64, ncols), dtype=np.float32)
        for r in range(4):
            m_pre[8 * ch_of_slot + r, col_of_slot] = g_dst[:, r]
            m_pre[8 * ch_of_slot + 4 + r, col_of_slot] = g_src[:, r]

        rel_pre = np.zeros((24, ncols), dtype=np.float32)
        relv = g_src[:, 4:7] - g_dst[:, 4:7]
        for r in range(3):
            rel_pre[3 * ch_of_slot + r, col_of_slot] = relv[:, r]

        em_col = s_all // TILE_E
        em_row = s_all % TILE_E
        keys_em = np.zeros((TILE_E, n_tiles_pad), dtype=np.float32)
        keys_em[em_row, em_col] = sl_key
        recip_em = np.zeros((TILE_E, n_tiles_pad), dtype=np.float32)
        recip_em[em_row, em_col] = sl_recip

        brel = np.full(nl_pad, GWIN - 1, dtype=np.uint8)
        brel[:nloc] = (batch[nlo : nlo + nloc] - c * G_PER_CORE).astype(np.uint8)
        brel_nm = np.zeros((128, nl_pad // 128), dtype=np.float32)
        brel_nm[np.arange(nl_pad) % 128, np.arange(nl_pad) // 128] = brel

        cu8 = np.zeros((128, BLK + GWIN + nl_pad // 128), dtype=np.float32)
        cu8[:, 0:BLK] = np.tile(np.arange(BLK, dtype=np.float32), (128, 1))
        cu8[:, BLK : BLK + GWIN] = np.tile(
            np.arange(GWIN, dtype=np.float32), (128, 1)
        )
        cu8[:, BLK + GWIN :] = brel_nm
        core_inputs.append(
            dict(
                m_pre=m_pre,
                rel_pre=rel_pre,
                keys_em=keys_em,
                recip_em=recip_em,
                const_u8=cu8,
            )
        )

    meta = dict(
        e_pad=e_pad,
        n_super=n_super,
        n_tiles_pad=n_tiles_pad,
        nl_pad=nl_pad,
        tile_block=tile_block,
        core_bounds=(core_node_lo, core_node_hi),
    )
    return core_inputs, meta


def _weight_inputs(W1, b1, W2, b2, Wp, bp, Wz, bz):
    W1 = np.asarray(W1, np.float32)
    W2 = np.asarray(W2, np.float32)
    W1xbd = np.zeros((64, 128), dtype=np.float32)
    W1dbd = np.zeros((8, 128), dtype=np.float32)
    W2bd = np.zeros((128, 128), dtype=np.float32)
    b1bd = np.zeros((128, 1), dtype=np.float32)
    b2bd = np.zeros((128, 1), dtype=np.float32)
    for c in range(NSTACK):
        W1xbd[8 * c : 8 * c + 8, 16 * c : 16 * c + 16] = W1[0:8]
        W1dbd[c, 16 * c : 16 * c + 16] = W1[8]
        W2bd[16 * c : 16 * c + 16, 16 * c : 16 * c + 16] = W2
        b1bd[16 * c : 16 * c + 16, 0] = b1
        b2bd[16 * c : 16 * c + 16, 0] = b2
    onesbd = np.zeros((24, 8), dtype=np.float32)
    for c in range(NSTACK):
        onesbd[3 * c : 3 * c + 3, c] = 1.0
    iota32 = np.tile(np.arange(BLK, dtype=np.uint8), (128, 1))
    iota_g = np.tile(np.arange(GWIN, dtype=np.uint8), (128, 1))
    Wzb = np.zeros((33, 8), dtype=np.float32)
    Wzb[:32] = np.asarray(Wz, np.float32)
    Wzb[32] = np.asarray(bz, np.float32)
    Wpd = np.asarray(Wp, np.float32)[:, 0:1] - np.asarray(Wp, np.float32)[:, 1:2]
    # pack: f32r block [128, 256+8]: W1x(0:128), W2(128:256), ones(256:264),
    #       W1d rows live in partitions 64:72 of W1x cols... keep separate cols:
    cr = np.zeros((128, 264 + 128), dtype=np.float32)
    cr[0:64, 0:128] = W1xbd
    cr[0:128, 128:256] = W2bd
    cr[0:24, 256:264] = onesbd
    cr[0:8, 264:392] = W1dbd
    cf = np.zeros((128, 2 + 1 + 8), dtype=np.float32)
    cf[:, 0:1] = b1bd
    cf[:, 1:2] = b2bd
    cf[0:16, 2:3] = Wpd
    cf[0:33, 3:11] = Wzb
    w = dict(const_f32r=cr, const_f32=cf)
    return w, float(np.asarray(bp)[0] - np.asarray(bp)[1])


def _build_program(meta, bp_diff):
    e_pad = meta["e_pad"]
    n_super = meta["n_super"]
    n_tiles_pad = meta["n_tiles_pad"]
    nl_pad = meta["nl_pad"]
    tile_block = meta["tile_block"]
    ncols = e_pad // NSTACK
    n_stripes = nl_pad // STRIPE
    n_ntile = nl_pad // 128

    f32, f32r, bf16, u8 = DT.float32, DT.float32r, DT.bfloat16, DT.uint8

    tile_stripe = tile_block // BLKS_PER_STRIPE
    stripe_last = (
        np.searchsorted(tile_stripe, np.arange(n_stripes), "right") - 1
    )

    nc = bacc.Bacc(None, target_bir_lowering=False, debug=False)

    d_m = nc.dram_tensor("m_pre", [64, ncols], f32r, kind="ExternalInput")
    d_rel = nc.dram_tensor("rel_pre", [24, ncols], f32, kind="ExternalInput")
    d_keys = nc.dram_tensor("keys_em", [TILE_E, n_tiles_pad], f32, kind="ExternalInput")
    d_recip = nc.dram_tensor(
        "recip_em", [TILE_E, n_tiles_pad], f32, kind="ExternalInput"
    )
    d_cr = nc.dram_tensor("const_f32r", [128, 392], f32r, kind="ExternalInput")
    d_cf = nc.dram_tensor("const_f32", [128, 11], f32, kind="ExternalInput")
    d_cu8 = nc.dram_tensor(
        "const_u8", [128, BLK + GWIN + n_ntile], f32, kind="ExternalInput"
    )

    d_s = nc.dram_tensor("s_out", [128, n_ntile, 2], f32, kind="ExternalOutput")
    d_z = nc.dram_tensor("z_out", [GWIN, 8], f32, kind="ExternalOutput")

    with tile.TileContext(nc) as tc, ExitStack() as ctx:
        const = ctx.enter_context(tc.tile_pool(name="const", bufs=1))
        sb = ctx.enter_context(tc.tile_pool(name="sb", bufs=3))
        sbe = ctx.enter_context(tc.tile_pool(name="sbe", bufs=4))
        sbn = ctx.enter_context(tc.tile_pool(name="sbn", bufs=2))
        persist = ctx.enter_context(tc.tile_pool(name="persist", bufs=1))
        ps_h1 = ctx.enter_context(tc.tile_pool(name="psh1", bufs=1, space="PSUM"))
        ps_msg = ctx.enter_context(tc.tile_pool(name="psmsg", bufs=1, space="PSUM"))
        ps_tr = ctx.enter_context(tc.tile_pool(name="pstr", bufs=2, space="PSUM"))
        ps_stripe = ctx.enter_context(tc.tile_pool(name="pss", bufs=2, space="PSUM"))
        ps_pool = ctx.enter_context(tc.tile_pool(name="psp", bufs=1, space="PSUM"))
        ps_misc = ctx.enter_context(tc.tile_pool(name="psm", bufs=1, space="PSUM"))

        c_r = const.tile([128, 392], f32r)
        c_f = const.tile([128, 11], f32)
        c_u = const.tile([128, BLK + GWIN + n_ntile], f32)
        nc.sync.dma_start(out=c_r[:], in_=d_cr[:])
        nc.sync.dma_start(out=c_f[:], in_=d_cf[:])
        nc.sync.dma_start(out=c_u[:], in_=d_cu8[:])
        cW1x = c_r[0:64, 0:128]
        cW2 = c_r[0:128, 128:256]
        cones = c_r[0:24, 256:264]
        cW1d = c_r[0:8, 264:392]
        cb1 = c_f[:, 0:1]
        cb2 = c_f[:, 1:2]
        cWp = c_f[0:16, 2:3]
        cWzb = c_f[0:33, 3:11]
        cio32 = c_u[:, 0:BLK]
        ciog = c_u[:, BLK : BLK + GWIN]
        cbrel = c_u[:, BLK + GWIN :]
        cident = const.tile([128, 128], f32)
        make_identity(nc, cident[:])
        czero16 = const.tile([1, 16], f32)
        czero512 = const.tile([1, CHUNK], f32)
        nc.gpsimd.memset(czero16[:], 0.0)
        nc.gpsimd.memset(czero512[:], 0.0)

        h_nm = persist.tile([128, n_ntile, 16], f32)
        s_nm = persist.tile([128, n_ntile, 2], f32)

        pooledT = ps_pool.tile([32, GWIN], f32)
        stripe_tiles = {}

        def open_stripe(s):
            pt = ps_stripe.tile([16, STRIPE], f32, tag="stripe")
            stripe_tiles[s] = pt
            empty = stripe_last[s] < 0 or (
                s > 0 and stripe_last[s] == stripe_last[s - 1]
            )
            nc.tensor.matmul(
                out=pt[:], lhsT=czero16[:], rhs=czero512[:],
                start=True, stop=bool(empty), skip_group_check=True,
            )

        def close_stripe(s):
            pt = stripe_tiles.pop(s)
            hT = sbn.tile([16, STRIPE], f32, tag="hT")
            nc.scalar.activation(out=hT[:], in_=pt[:], func=AF.Relu)
            node_phase(s, hT)

        def node_phase(s, hT):
            lg = ps_misc.tile([1, STRIPE], f32, tag="misc")
            nc.tensor.matmul(
                out=lg[:], lhsT=cWp, rhs=hT[:],
                start=True, stop=True,
            )
            s0 = sbn.tile([1, STRIPE], f32, tag="s0")
            nc.scalar.activation(
                out=s0[:], in_=lg[:], func=AF.Sigmoid, bias=float(bp_diff)
            )
            for j in range(STRIPE // 128):
                nt = s * (STRIPE // 128) + j
                ph = ps_misc.tile([128, 16], f32, tag="misc")
                nc.tensor.transpose(
                    out=ph[:], in_=hT[:, j * 128 : (j + 1) * 128],
                    identity=cident[:16, :16],
                )
                nc.vector.tensor_copy(out=h_nm[:, nt, :], in_=ph[:])
                psn = ps_misc.tile([128, 1], f32, tag="misc")
                nc.tensor.transpose(
                    out=psn[:], in_=s0[:, j * 128 : (j + 1) * 128],
                    identity=cident[:1, :1],
                )
                nc.vector.tensor_copy(out=s_nm[:, nt, 0:1], in_=psn[:])
                nc.scalar.activation(
                    out=s_nm[:, nt, 1:2], in_=s_nm[:, nt, 0:1],
                    func=AF.Copy, scale=-1.0, bias=1.0,
                )
                outer = sbn.tile([128, 32], bf16, tag="outer")
                nc.vector.tensor_tensor(
                    out=outer[:, 0:16],
                    in0=h_nm[:, nt, :],
                    in1=s_nm[:, nt, 0:1].to_broadcast([128, 16]),
                    op=ALU.mult,
                )
                nc.vector.tensor_tensor(
                    out=outer[:, 16:32],
                    in0=h_nm[:, nt, :],
                    in1=s_nm[:, nt, 1:2].to_broadcast([128, 16]),
                    op=ALU.mult,
                )
                og = sbn.tile([128, GWIN], bf16, tag="og")
                nc.vector.tensor_tensor(
                    out=og[:],
                    in0=cbrel[:, nt : nt + 1].to_broadcast([128, GWIN]),
                    in1=ciog,
                    op=ALU.is_equal,
                )
                nc.tensor.matmul(
                    out=pooledT[:], lhsT=outer[:], rhs=og[:],
                    start=(nt == 0), stop=(nt == n_ntile - 1),
                    skip_group_check=True,
                )

        # ---------------- edge phase ----------------
        cur_stripe = 0
        open_stripe(0)

        for sp in range(n_super):
            mt = sb.tile([64, CHUNK], f32r, tag="m")
            nc.sync.dma_start(out=mt[:], in_=d_m[:, sp * CHUNK : (sp + 1) * CHUNK])
            rt = sb.tile([24, CHUNK], f32, tag="rel")
            nc.sync.dma_start(
                out=rt[:], in_=d_rel[:, sp * CHUNK : (sp + 1) * CHUNK]
            )
            kt = sb.tile([TILE_E, TILES_PER_SUPER], f32, tag="keys")
            nc.sync.dma_start(
                out=kt[:],
                in_=d_keys[:, sp * TILES_PER_SUPER : (sp + 1) * TILES_PER_SUPER],
            )
            rct = sb.tile([TILE_E, TILES_PER_SUPER], f32, tag="recip")
            nc.sync.dma_start(
                out=rct[:],
                in_=d_recip[:, sp * TILES_PER_SUPER : (sp + 1) * TILES_PER_SUPER],
            )

            sq = sb.tile([24, CHUNK], f32r, tag="sq")
            nc.vector.tensor_tensor(out=sq[:], in0=rt[:], in1=rt[:], op=ALU.mult)
            d2 = ps_misc.tile([8, CHUNK], f32, tag="misc")
            nc.tensor.matmul(
                out=d2[:], lhsT=cones, rhs=sq[:],
                start=True, stop=True,
            )
            dist = sb.tile([8, CHUNK], f32r, tag="dist")
            nc.scalar.activation(out=dist[:], in_=d2[:], func=AF.Sqrt)

            h1p = ps_h1.tile([128, CHUNK], f32, tag="h1")
            nc.tensor.matmul(
                out=h1p[:], lhsT=cW1x, rhs=mt[:],
                start=True, stop=False, skip_group_check=True,
            )
            nc.tensor.matmul(
                out=h1p[:], lhsT=cW1d, rhs=dist[:],
                start=False, stop=True, skip_group_check=True,
            )
            h1 = sb.tile([128, CHUNK], f32r, tag="h1s")
            nc.scalar.activation(out=h1[:], in_=h1p[:], func=AF.Silu, bias=cb1)
            msgp = ps_msg.tile([128, CHUNK], f32, tag="msg")
            nc.tensor.matmul(
                out=msgp[:], lhsT=cW2, rhs=h1[:],
                start=True, stop=True,
            )
            msgT = sb.tile([128, CHUNK], f32, tag="msgT")
            nc.vector.tensor_tensor(
                out=msgT[:], in0=msgp[:],
                in1=cb2.to_broadcast([128, CHUNK]), op=ALU.add,
            )

            oh = sb.tile([TILE_E, TILES_PER_SUPER, BLK], bf16, tag="oh")
            nc.vector.tensor_tensor(
                out=oh[:],
                in0=kt[:].unsqueeze(2).to_broadcast(
                    [TILE_E, TILES_PER_SUPER, BLK]
                ),
                in1=cio32.unsqueeze(1).to_broadcast(
                    [TILE_E, TILES_PER_SUPER, BLK]
                ),
                op=ALU.is_equal,
            )

            msge_of_b = []
            for b in range(4):
                trp = ps_tr.tile([128, 128], f32, tag="tr")
                nc.tensor.transpose(
                    out=trp[:], in_=msgT[:, b * 128 : (b + 1) * 128],
                    identity=cident[:],
                )
                msge = sbe.tile([128, NSTACK, 16], bf16, tag="msge")
                nc.vector.tensor_tensor(
                    out=msge[:],
                    in0=trp[:].rearrange("p (a b) -> p a b", a=NSTACK),
                    in1=rct[:, b : TILES_PER_SUPER : 4]
                    .unsqueeze(2)
                    .to_broadcast([TILE_E, NSTACK, 16]),
                    op=ALU.mult,
                )
                msge_of_b.append(msge)

            for cch in range(NSTACK):
                for b in range(4):
                    t_loc = cch * 4 + b
                    t_glob = sp * TILES_PER_SUPER + t_loc
                    s_of_t = int(tile_stripe[t_glob])
                    while cur_stripe < s_of_t:
                        close_stripe(cur_stripe)
                        cur_stripe += 1
                        open_stripe(cur_stripe)
                    blk = int(tile_block[t_glob])
                    coff = (blk % BLKS_PER_STRIPE) * BLK
                    pt = stripe_tiles[cur_stripe]
                    nc.tensor.matmul(
                        out=pt[:, coff : coff + BLK],
                        lhsT=msge_of_b[b][:, cch, :],
                        rhs=oh[:, t_loc, :],
                        start=False,
                        stop=bool(t_glob == int(stripe_last[s_of_t])),
                        skip_group_check=True,
                    )

        close_stripe(cur_stripe)
        for s2 in range(cur_stripe + 1, n_stripes):
            open_stripe(s2)
            close_stripe(s2)

        # ---------------- outputs ----------------
        pooled_sb = sbn.tile([33, GWIN], f32, tag="pooled")
        nc.gpsimd.memset(pooled_sb[:], 1.0)
        nc.vector.tensor_copy(out=pooled_sb[0:32, :], in_=pooledT[:])
        zp = ps_misc.tile([128, 8], f32, tag="misc")
        nc.tensor.matmul(
            out=zp[:],
            lhsT=pooled_sb[:, 0:128],
            rhs=cWzb,
            start=True, stop=True,
        )
        zs = sbn.tile([128, 8], f32, tag="zs")
        nc.vector.tensor_copy(out=zs[:], in_=zp[:])
        nc.sync.dma_start(out=d_z[0:128, :], in_=zs[:])
        zp2 = ps_misc.tile([32, 8], f32, tag="misc")
        nc.tensor.matmul(
            out=zp2[:],
            lhsT=pooled_sb[:, 128:GWIN],
            rhs=cWzb,
            start=True, stop=True,
        )
        zs2 = sbn.tile([32, 8], f32, tag="zs2")
        nc.vector.tensor_copy(out=zs2[:], in_=zp2[:])
        nc.sync.dma_start(out=d_z[128:GWIN, :], in_=zs2[:])

        nc.sync.dma_start(out=d_s[:], in_=s_nm[:])

    nc.compile()
    return nc


LAST_RESULT = None
LAST_EXEC_WALL_S = None


def kernel(x, pos, W1, b1, W2, b2, Wp, bp, Wz, bz, edge_index, batch):
    x = np.asarray(x, dtype=np.float32)
    pos = np.asarray(pos, dtype=np.float32)

    core_inputs, meta = _host_prep(x, pos, edge_index, batch)
    weights, bp_diff = _weight_inputs(W1, b1, W2, b2, Wp, bp, Wz, bz)

    nc = _build_program(meta, bp_diff)

    in_maps = []
    for c in range(N_CORES):
        m = dict(core_inputs[c])
        m.update(weights)
        in_maps.append(m)

    import time as _time

    t0 = _time.time()
    res = run_bass_kernel_spmd(nc, in_maps, list(range(N_CORES)))
    global LAST_RESULT, LAST_EXEC_WALL_S
    LAST_RESULT = res
    LAST_EXEC_WALL_S = _time.time() - t0
    results = res.results

    core_node_lo, core_node_hi = meta["core_bounds"]
    s_full = np.zeros((N_NODES, 2), dtype=np.float32)
    z_full = np.zeros((N_GRAPHS, 8), dtype=np.float32)
    for c in range(N_CORES):
        nlo, nhi = int(core_node_lo[c]), int(core_node_hi[c])
        nloc = nhi - nlo
        s_pc = results[c]["s_out"].transpose(1, 0, 2).reshape(-1, 2)
        s_full[nlo:nhi] = s_pc[:nloc]
        z_full[c * G_PER_CORE : (c + 1) * G_PER_CORE] = results[c]["z_out"][
            :G_PER_CORE
        ]
    return z_full, s_full
